# revision 9
# baseline (speedup 1.0000x reference)
"""Causal multi-head attention block (B=4, T=2048, C=1024, H=16) on 8 NeuronCores.

Sharding: core c = 2*b + hg handles batch b, head-group hg (8 heads).
Data parallel over B, tensor parallel over heads: qkv weights column-split,
proj weights row-split; each core emits a partial projection output which the
host sums per batch (plus proj bias).

v2 pipeline (single interleaved PE stream, heads processed in PAIRS):
  - q/k strips qkT[jt] [128, 2048] (fp32r, head dims on partitions; strip jt
    holds heads 2jt / 2jt+1 on partition halves).  Pair p uses jq=p, jk=4+p.
  - scores: ROW-TILED pair matmuls (K=64): lhsT = qkT[jk][0:64]/[64:128],
    rhs = qkT[jq][same rows] -> two concurrent MMs on array row halves.
  - exp on ACT per (head, cj) over [128, <=1024] psum -> bf16 expT in SBUF.
  - AV: COL-TILED pair matmuls (M=64): yT joint psum tile [128, 1024] with
    head-even y on partitions 0:64, head-odd on 64:128.
  - softmax sums: separate 1-col matmuls vs a ones vector, 4x col-tiled into
    one psum bank (partition 32*(2*head_lo + query_quarter)).
  - normalization: sums -> DVE copy -> DRAM bounce -> wide reciprocal ->
    broadcast -> one [128, 1024] DVE mul writing bf16 yT_sb.
  - qkT strips for pair p+1, the v projection, and P4 tiles are interleaved
    into P2's exp-wait gaps as PE fillers (PSUM: sc 4 + yT 2 + sums 1 +
    fill 1 = 8 banks).  PE instruction queue is in-order, so fillers are
    emitted BEFORE the dependency-stalled attention MMs of each cj window.
  - P4: yT_sb (bf16) x wp (bf16) partial projection, host sums core pairs.
"""
import numpy as np
from collections import deque

import ml_dtypes

import concourse.bacc as bacc
import concourse.mybir as mybir
import concourse.tile as tile
from concourse.bass_utils import run_bass_kernel_spmd

B, T, C, H, D = 4, 2048, 1024, 16, 64
NC_CORES = 8
HPC = H // 2          # heads per core = 8
NPAIR = HPC // 2      # head pairs per core = 4
CW = 3 * C // 2       # packed local qkv width = 1536
NT = T // 128         # 16 token tiles
NCC = C // 128        # 8 contraction chunks
HALF = T // 2
F32 = mybir.dt.float32
F32R = mybir.dt.float32r
BF16 = mybir.dt.bfloat16

TRACE = False          # test.py sets True to profile
LAST_RESULT = None     # BassKernelResults of the last run (for test.py)

_cached_nc = None


def _build():
    global _cached_nc
    if _cached_nc is not None:
        return _cached_nc

    nc = bacc.Bacc("TRN2", debug=False)

    xT_d = nc.dram_tensor("xT", [C, T], F32R, kind="ExternalInput")
    w_d = nc.dram_tensor("w", [C, CW], F32R, kind="ExternalInput")
    wp_d = nc.dram_tensor("wp", [C // 2, C], BF16, kind="ExternalInput")
    bqk_d = nc.dram_tensor("bqk", [128, 8], F32, kind="ExternalInput")
    bv_d = nc.dram_tensor("bv", [128, 512], F32, kind="ExternalInput")
    mask01_d = nc.dram_tensor("mask01", [128, 128], F32, kind="ExternalInput")
    out_d = nc.dram_tensor("partial", [T, C], F32, kind="ExternalOutput")

    with tile.TileContext(nc) as tc:
        with (
            tc.tile_pool(name="const", bufs=1) as const,
            tc.tile_pool(name="xp", bufs=1) as xp,
            tc.tile_pool(name="qkp", bufs=5) as qkp,
            tc.tile_pool(name="wvp", bufs=1) as wvp,
            tc.tile_pool(name="wjp", bufs=3) as wjp,
            tc.tile_pool(name="expp", bufs=3) as expp,
            tc.tile_pool(name="nrm", bufs=2) as nrm,
            tc.tile_pool(name="osbp", bufs=3) as osbp,
            tc.tile_pool(name="dramp", bufs=4, space="DRAM") as dramp,
        ):
            # ---- constants ----
            mask01f = const.tile([128, 128], F32)
            nc.sync.dma_start(mask01f[:], mask01_d.ap())
            mask01 = const.tile([128, 128], BF16)
            nc.vector.tensor_copy(mask01[:], mask01f[:])
            bqk = const.tile([128, 8], F32)
            nc.sync.dma_start(bqk[:], bqk_d.ap())
            bv = const.tile([128, 512], F32)
            nc.sync.dma_start(bv[:], bv_d.ap())
            ones_sb = const.tile([128, 1], BF16)
            nc.vector.memset(ones_sb[:], 1.0)
            v_aug = const.tile([128, NT, HPC, D], BF16)
            yT_sb = [const.tile([128, T], BF16, name=f"yT{k}") for k in range(NPAIR)]
            wp_sb = const.tile([128, 4, C], BF16)

            # qkT strip tiles, allocated lazily (rotation bufs=5)
            qkT = [None] * 8

            # ---- input DMA order: pair-0 weights first, then xT chunks ----
            w_p0 = []
            for jt in (0, 4):
                w_jt = wjp.tile([128, NCC, 128], F32R, tag="wjt", name=f"w{jt}")
                nc.sync.dma_start(
                    w_jt[:],
                    w_d.ap()[:, jt * 128:(jt + 1) * 128].rearrange(
                        "(cc p) j -> p cc j", p=128
                    ),
                )
                w_p0.append(w_jt)
            xT = [xp.tile([128, T], F32R, name=f"xT{i}") for i in range(NCC)]
            for ccu in range(6):
                nc.sync.dma_start(xT[ccu][:], xT_d.ap()[ccu * 128:(ccu + 1) * 128, :])
            wv = wvp.tile([128, NCC, 512], F32R)
            nc.sync.dma_start(
                wv[:], w_d.ap()[:, 1024:1536].rearrange("(cc p) j -> p cc j", p=128)
            )
            for ccu in range(6, NCC):
                nc.sync.dma_start(xT[ccu][:], xT_d.ap()[ccu * 128:(ccu + 1) * 128, :])

            # ---- P1a: pair-0 q/k strips, trickled behind the xT DMAs ----
            with tc.tile_pool(name="ps1", bufs=1, space="PSUM") as ps1:
                ps_strip = [
                    ps1.tile([128, T], F32, tag=f"s{si}", name=f"ps_s{si}")
                    for si in range(2)
                ]
                # warm the PE clock gate with tiny bf16 matmuls before xT lands
                for wi in range(16):
                    nc.tensor.matmul(
                        ps_strip[0][:, 0:128],
                        lhsT=mask01[:],
                        rhs=mask01[:],
                        start=True,
                        stop=True,
                        skip_group_check=True,
                    )
                for cc in range(NCC):
                    for si in range(2):
                        for tck in range(4):
                            nc.tensor.matmul(
                                ps_strip[si][:, tck * 512:(tck + 1) * 512],
                                lhsT=w_p0[si][:, cc, :],
                                rhs=xT[cc][:, tck * 512:(tck + 1) * 512],
                                start=(cc == 0),
                                stop=(cc == NCC - 1),
                            )
                for si, jt in ((0, 0), (1, 4)):
                    q = qkp.tile([128, T], F32R, tag="qkT", name=f"qkT{jt}")
                    qkT[jt] = q
                    for tck in range(4):
                        nc.scalar.activation(
                            q[:, tck * 512:(tck + 1) * 512],
                            ps_strip[si][:, tck * 512:(tck + 1) * 512],
                            mybir.ActivationFunctionType.Identity,
                            bias=bqk[:, jt:jt + 1],
                        )

            # ---- main psum pool: sc pair 4 + yT 2 + sums 1 + fill 1 ----
            with tc.tile_pool(name="ps2", bufs=1, space="PSUM") as ps2:
                # ---- PE filler generators (yield after each MM) ----
                progress = {"v": -1}  # last fully-emitted v token tile
                strip_done = set()    # pairs whose q/k strips are fully emitted

                def v_gen():
                    """v projection: token-major, bias + bf16 into v_aug."""
                    for tt in range(NT):
                        ps = ps2.tile([128, 512], F32, tag="fill", name="vps")
                        for cc in range(NCC):
                            nc.tensor.matmul(
                                ps[:],
                                lhsT=xT[cc][:, tt * 128:(tt + 1) * 128],
                                rhs=wv[:, cc, :],
                                start=(cc == 0),
                                stop=(cc == NCC - 1),
                            )
                            yield
                        nc.vector.tensor_add(
                            v_aug[:, tt, :, :],
                            ps[:].rearrange("p (h d) -> p h d", h=HPC),
                            bv[:].rearrange("p (h d) -> p h d", h=HPC),
                        )
                        progress["v"] = tt
                        yield

                def strip_gen(p):
                    """q/k strips for pair p (jt = p and 4+p), 1 psum bank."""
                    for jt in (p, 4 + p):
                        w_jt = wjp.tile(
                            [128, NCC, 128], F32R, tag="wjt", name=f"w{jt}"
                        )
                        nc.sync.dma_start(
                            w_jt[:],
                            w_d.ap()[:, jt * 128:(jt + 1) * 128].rearrange(
                                "(cc p) j -> p cc j", p=128
                            ),
                        )
                        q = qkp.tile([128, T], F32R, tag="qkT", name=f"qkT{jt}")
                        qkT[jt] = q
                        for tck in range(4):
                            ps = ps2.tile([128, 512], F32, tag="fill", name="sps")
                            for cc in range(NCC):
                                nc.tensor.matmul(
                                    ps[:],
                                    lhsT=w_jt[:, cc, :],
                                    rhs=xT[cc][:, tck * 512:(tck + 1) * 512],
                                    start=(cc == 0),
                                    stop=(cc == NCC - 1),
                                )
                                yield
                            nc.vector.tensor_scalar_add(
                                q[:, tck * 512:(tck + 1) * 512],
                                ps[:],
                                bqk[:, jt:jt + 1],
                            )
                            yield
                    strip_done.add(p)

                def p4_gen(ci_lo, ci_hi, tag):
                    """output projection tiles: [128, 512] psum groups."""
                    for ci in range(ci_lo, ci_hi):
                        for nck in range(2):
                            po = ps2.tile([128, 512], F32, tag=tag, name="po")
                            for ki, kc in enumerate((3, 0, 1, 2)):
                                nc.tensor.matmul(
                                    po[:],
                                    lhsT=yT_sb[kc][:, ci * 128:(ci + 1) * 128],
                                    rhs=wp_sb[:, kc, nck * 512:(nck + 1) * 512],
                                    start=(ki == 0),
                                    stop=(ki == 3),
                                )
                                yield
                            o = osbp.tile([128, 512], F32, tag="osb", name="osb")
                            nc.vector.tensor_copy(o[:], po[:])
                            nc.sync.dma_start(
                                out_d.ap()[
                                    ci * 128:(ci + 1) * 128,
                                    nck * 512:(nck + 1) * 512,
                                ],
                                o[:],
                            )
                            yield

                fillers = deque([v_gen(), strip_gen(1), strip_gen(2), strip_gen(3)])

                def pump(n):
                    while n > 0 and fillers:
                        try:
                            next(fillers[0])
                            n -= 1
                        except StopIteration:
                            fillers.popleft()

                def drain_fillers():
                    while fillers:
                        try:
                            next(fillers[0])
                        except StopIteration:
                            fillers.popleft()

                def pump_until(cond):
                    while not cond() and fillers:
                        try:
                            next(fillers[0])
                        except StopIteration:
                            fillers.popleft()

                # ---- P2: attention per pair ----
                for p in range(NPAIR):
                    if p > 0:
                        # pair p's q/k strips must be fully EMITTED before any
                        # scores MM reads them (in-order emission = program
                        # order; a late strip write after an early read would
                        # read stale data)
                        pump_until(lambda: p in strip_done)
                    jq, jk = p, 4 + p
                    kT, qT = qkT[jk], qkT[jq]
                    for half in range(2):
                        if p == NPAIR - 1 and half == 1:
                            # safe only now: these read yT_sb[3] half 0,
                            # whose norm was emitted at the end of half 0
                            fillers.append(p4_gen(0, 8, "fill"))
                        ilo, ihi = half * HALF, (half + 1) * HALF
                        cjmax = ihi // 128
                        yT_ps = ps2.tile([128, HALF], F32, tag="yT", name="yTps")
                        sums_ps = ps2.tile([128, 512], F32, tag="sums", name="sums")

                        def emit_scores_exp(cj):
                            i0 = cj * 128
                            s0 = max(i0, ilo)
                            scA = ps2.tile([128, HALF], F32, tag="scA", name="scA")
                            scB = ps2.tile([128, HALF], F32, tag="scB", name="scB")
                            s = s0
                            while s < ihi:
                                e = min(s + 512 - s % 512, ihi)
                                for rp, sc in ((0, scA), (64, scB)):
                                    nc.tensor.matmul(
                                        sc[:, s - ilo:e - ilo],
                                        lhsT=kT[rp:rp + 64, i0:i0 + 128],
                                        rhs=qT[rp:rp + 64, s:e],
                                        start=True,
                                        stop=True,
                                        skip_group_check=True,
                                    )
                                s = e
                            exps = []
                            for hb, sc in ((0, scA), (1, scB)):
                                ex = expp.tile(
                                    [128, HALF], BF16, tag=f"exp{hb}", name=f"ex{hb}"
                                )
                                nc.scalar.activation(
                                    ex[:, :ihi - s0],
                                    sc[:, s0 - ilo:ihi - ilo],
                                    mybir.ActivationFunctionType.Exp,
                                )
                                if i0 >= ilo:
                                    nc.vector.tensor_mul(
                                        ex[:, 0:128], ex[:, 0:128], mask01[:]
                                    )
                                exps.append(ex)
                            return exps

                        def emit_av_sums(cj, exps):
                            i0 = cj * 128
                            s0 = max(i0, ilo)
                            s = s0
                            while s < ihi:
                                e = min(s + 512 - s % 512, ihi)
                                for hb, ex in ((0, exps[0]), (1, exps[1])):
                                    nc.tensor.matmul(
                                        yT_ps[hb * 64:hb * 64 + 64, s - ilo:e - ilo],
                                        lhsT=v_aug[:, cj, 2 * p + hb, :],
                                        rhs=ex[:, s - s0:e - s0],
                                        start=(cj == 0),
                                        stop=(cj == cjmax - 1),
                                        skip_group_check=True,
                                        tile_position=(0, hb * 64),
                                    )
                                qtr = (s - ilo) // 512
                                for hb, ex in ((0, exps[0]), (1, exps[1])):
                                    rp = 32 * (2 * hb + qtr)
                                    nc.tensor.matmul(
                                        sums_ps[rp:rp + 1,
                                                s - ilo - qtr * 512:
                                                e - ilo - qtr * 512],
                                        lhsT=ones_sb[:],
                                        rhs=ex[:, s - s0:e - s0],
                                        start=(cj == 0),
                                        stop=(cj == cjmax - 1),
                                        skip_group_check=True,
                                        tile_position=(0, rp),
                                    )
                                s = e

                        prev = None
                        for cj in range(cjmax):
                            pump(3)
                            if prev is not None:
                                # AV(cj-1) reads v_aug[:, cj-1]; make sure the
                                # v filler has emitted that token tile
                                pump_until(
                                    lambda: progress["v"] >= prev[0]
                                )
                                emit_av_sums(*prev)
                            prev = (cj, emit_scores_exp(cj))
                        pump_until(lambda: progress["v"] >= prev[0])
                        emit_av_sums(*prev)
                        if p == NPAIR - 1 and half == 1:
                            drain_fillers()

                        # ---- normalization ----
                        sums_sb = nrm.tile([128, 512], F32, tag="ssb", name="ssb")
                        nc.vector.tensor_copy(sums_sb[:], sums_ps[:])
                        s_dram = dramp.tile([1, 2 * HALF], F32, tag="sd")
                        for r in range(4):
                            nc.gpsimd.dma_start(
                                s_dram[:, r * 512:(r + 1) * 512],
                                sums_sb[32 * r:32 * r + 1, :],
                            )
                        srb = nrm.tile([128, 16], F32, tag="srb", name="srb")
                        nc.gpsimd.dma_start(
                            srb[:], s_dram[:].rearrange("o (p f) -> (o p) f", p=128)
                        )
                        rcp = nrm.tile([128, 16], F32, tag="rcp", name="rcp")
                        nc.vector.reciprocal(rcp[:], srb[:])
                        r_dram = dramp.tile([1, 2 * HALF], F32, tag="rd")
                        nc.gpsimd.dma_start(
                            r_dram[:].rearrange("o (p f) -> (o p) f", p=128), rcp[:]
                        )
                        rb_sb = nrm.tile([128, HALF], F32, tag="rb", name="rb")
                        nc.gpsimd.dma_start(
                            rb_sb[0:64, :],
                            r_dram[:, 0:HALF].to_broadcast((64, HALF)),
                        )
                        nc.gpsimd.dma_start(
                            rb_sb[64:128, :],
                            r_dram[:, HALF:2 * HALF].to_broadcast((64, HALF)),
                        )
                        nc.vector.tensor_mul(
                            yT_sb[p][:, ilo:ihi], yT_ps[:], rb_sb[:]
                        )
                        if p == 0 and half == 0:
                            # wp load once the input-DMA burst has quieted
                            nc.sync.dma_start(
                                wp_sb[:],
                                wp_d.ap().rearrange("(kc p) n -> p kc n", p=128),
                            )

                # ---- P4 remainder: round-robin two psum tags ----
                gens = deque([p4_gen(8, 12, "scA"), p4_gen(12, 16, "scB")])
                while gens:
                    g = gens.popleft()
                    try:
                        next(g)
                        gens.append(g)
                    except StopIteration:
                        pass

    nc.compile()
    _cached_nc = nc
    return nc


def kernel(x, attn_w, attn_b, proj_w, proj_b):
    global LAST_RESULT
    x = np.asarray(x, dtype=np.float32)
    attn_w = np.asarray(attn_w, dtype=np.float32)
    attn_b = np.asarray(attn_b, dtype=np.float32)
    proj_w = np.asarray(proj_w, dtype=np.float32)
    proj_b = np.asarray(proj_b, dtype=np.float32)

    nc = _build()

    mask01 = np.triu(np.ones((128, 128), dtype=np.float32))  # keep j<=i
    in_maps = []
    for core in range(NC_CORES):
        b, hg = core // 2, core % 2
        qs = slice(hg * 512, hg * 512 + 512)
        ks = slice(C + hg * 512, C + hg * 512 + 512)
        vs = slice(2 * C + hg * 512, 2 * C + hg * 512 + 512)
        w_c = np.concatenate(
            [attn_w[:, qs], attn_w[:, ks], attn_w[:, vs]], axis=1
        )
        in_maps.append(
            {
                "xT": np.ascontiguousarray(x[b].T),
                "w": np.ascontiguousarray(w_c),
                "wp": np.ascontiguousarray(
                    proj_w[hg * 512:hg * 512 + 512, :]
                ).astype(ml_dtypes.bfloat16),
                "bqk": np.ascontiguousarray(
                    np.concatenate([attn_b[qs], attn_b[ks]]).reshape(8, 128).T
                ),
                "bv": np.ascontiguousarray(
                    np.broadcast_to(attn_b[vs][None, :], (128, 512))
                ),
                "mask01": mask01,
            }
        )

    res = run_bass_kernel_spmd(
        nc, in_maps, core_ids=list(range(NC_CORES)), trace=TRACE
    )
    LAST_RESULT = res

    out = np.empty((B, T, C), dtype=np.float32)
    for b in range(B):
        out[b] = (
            res.results[2 * b]["partial"]
            + res.results[2 * b + 1]["partial"]
            + proj_b[None, :]
        )
    return out


# revision 13
# speedup vs baseline: 1.0254x; 1.0254x over previous
"""Causal multi-head attention block (B=4, T=2048, C=1024, H=16) on 8 NeuronCores.

Sharding: core c = 2*b + hg handles batch b, head-group hg (8 heads).
Data parallel over B, tensor parallel over heads: qkv weights column-split,
proj weights row-split; each core emits a partial projection output which the
host sums per batch (plus proj bias).

v3 = baseline P2 matmul structure (K=128 zero-padded scores, M=65 ones-column
AV -- LDWEIGHTS fully hidden, 227ns issue pace) + interleaved PE scheduling:
  - pair-0 q/k strips trickle behind the xT input DMAs (PE starts ~5us);
  - the v projection, later q/k strips, and the first half of P4 are PE
    "fillers" pumped into P2's exp-wait gaps (in-order PE queue: fillers are
    emitted BEFORE the dependency-stalled attention MMs of each cj window);
  - normalization bounces ride the idle gpsimd DMA queue; the scale-mul reads
    yT straight from PSUM; yT_sb and wp are bf16 (P4 in bf16).
PSUM: sc [128,1024] x2bufs (4 banks) + yT [65,1024] (2) + fill [128,512]
x2bufs (2) = 8 banks.
"""
import numpy as np
from collections import deque

import ml_dtypes

import concourse.bacc as bacc
import concourse.mybir as mybir
import concourse.tile as tile
from concourse.bass_utils import run_bass_kernel_spmd

B, T, C, H, D = 4, 2048, 1024, 16, 64
NC_CORES = 8
HPC = H // 2          # heads per core = 8
NPAIR = HPC // 2      # head pairs per core = 4
CW = 3 * C // 2       # packed local qkv width = 1536
NT = T // 128         # 16 token tiles
NCC = C // 128        # 8 contraction chunks
HALF = T // 2
F32 = mybir.dt.float32
F32R = mybir.dt.float32r
BF16 = mybir.dt.bfloat16

TRACE = False          # test.py sets True to profile
LAST_RESULT = None     # BassKernelResults of the last run (for test.py)

_cached_nc = None


def _build():
    global _cached_nc
    if _cached_nc is not None:
        return _cached_nc

    nc = bacc.Bacc("TRN2", debug=False)

    xT_d = nc.dram_tensor("xT", [C, T], F32R, kind="ExternalInput")
    w_d = nc.dram_tensor("w", [C, CW], F32R, kind="ExternalInput")
    wp_d = nc.dram_tensor("wp", [C // 2, C], BF16, kind="ExternalInput")
    bqk_d = nc.dram_tensor("bqk", [128, 8], F32, kind="ExternalInput")
    bv_d = nc.dram_tensor("bv", [128, 512], F32, kind="ExternalInput")
    mask01_d = nc.dram_tensor("mask01", [128, 128], F32, kind="ExternalInput")
    out_d = nc.dram_tensor("partial", [T, C], F32, kind="ExternalOutput")

    with tile.TileContext(nc) as tc:
        with (
            tc.tile_pool(name="const", bufs=1) as const,
            tc.tile_pool(name="xp", bufs=1) as xp,
            tc.tile_pool(name="qkp", bufs=5) as qkp,
            tc.tile_pool(name="wvp", bufs=1) as wvp,
            tc.tile_pool(name="wjp", bufs=2) as wjp,
            tc.tile_pool(name="expp", bufs=3) as expp,
            tc.tile_pool(name="nrm", bufs=1) as nrm,
            tc.tile_pool(name="osbp", bufs=3) as osbp,
            tc.tile_pool(name="dramp", bufs=4, space="DRAM") as dramp,
        ):
            # ---- constants ----
            mask01f = const.tile([128, 128], F32)
            nc.sync.dma_start(mask01f[:], mask01_d.ap())
            mask01 = const.tile([128, 128], BF16)
            nc.vector.tensor_copy(mask01[:], mask01f[:])
            bqk = const.tile([128, 8], F32)
            nc.sync.dma_start(bqk[:], bqk_d.ap())
            bv = const.tile([128, 512], F32)
            nc.sync.dma_start(bv[:], bv_d.ap())
            v_aug = const.tile([128, NT, HPC, D + 1], BF16)
            yT_sb = [const.tile([128, T], BF16, name=f"yT{k}") for k in range(NPAIR)]
            wp_sb = const.tile([128, 4, C], BF16)
            kpad = [const.tile([128, T], F32R, name=f"kpad{s}") for s in range(2)]
            nc.vector.memset(kpad[0][:].bitcast(F32), 0.0)
            nc.vector.memset(kpad[1][:].bitcast(F32), 0.0)
            nc.vector.memset(v_aug[:, :, :, D:D + 1], 1.0)

            # qkT strip tiles, allocated lazily (rotation bufs=5)
            qkT = [None] * 8

            # ---- input DMA order: pair-0 weights first, then xT chunks ----
            w_p0 = []
            for jt in (0, 4):
                w_jt = wjp.tile([128, NCC, 128], F32R, tag="wjt", name=f"w{jt}")
                nc.sync.dma_start(
                    w_jt[:],
                    w_d.ap()[:, jt * 128:(jt + 1) * 128].rearrange(
                        "(cc p) j -> p cc j", p=128
                    ),
                )
                w_p0.append(w_jt)
            xT = [xp.tile([128, T], F32R, name=f"xT{i}") for i in range(NCC)]
            for ccu in range(6):
                nc.sync.dma_start(xT[ccu][:], xT_d.ap()[ccu * 128:(ccu + 1) * 128, :])
            wv = wvp.tile([128, NCC, 512], F32R)
            nc.sync.dma_start(
                wv[:], w_d.ap()[:, 1024:1536].rearrange("(cc p) j -> p cc j", p=128)
            )
            for ccu in range(6, NCC):
                nc.sync.dma_start(xT[ccu][:], xT_d.ap()[ccu * 128:(ccu + 1) * 128, :])

            # ---- P1a: pair-0 q/k strips, trickled behind the xT DMAs ----
            with tc.tile_pool(name="ps1", bufs=1, space="PSUM") as ps1:
                ps_strip = [
                    ps1.tile([128, T], F32, tag=f"s{si}", name=f"ps_s{si}")
                    for si in range(2)
                ]
                for wi in range(16):
                    nc.tensor.matmul(
                        ps_strip[0][:, 0:128],
                        lhsT=mask01[:],
                        rhs=mask01[:],
                        start=True,
                        stop=True,
                        skip_group_check=True,
                    )
                for cc in range(NCC):
                    for si in range(2):
                        for tck in range(4):
                            nc.tensor.matmul(
                                ps_strip[si][:, tck * 512:(tck + 1) * 512],
                                lhsT=w_p0[si][:, cc, :],
                                rhs=xT[cc][:, tck * 512:(tck + 1) * 512],
                                start=(cc == 0),
                                stop=(cc == NCC - 1),
                            )
                for si, jt in ((0, 0), (1, 4)):
                    q = qkp.tile([128, T], F32R, tag="qkT", name=f"qkT{jt}")
                    qkT[jt] = q
                    for tck in range(4):
                        nc.scalar.activation(
                            q[:, tck * 512:(tck + 1) * 512],
                            ps_strip[si][:, tck * 512:(tck + 1) * 512],
                            mybir.ActivationFunctionType.Identity,
                            bias=bqk[:, jt:jt + 1],
                        )

            # ---- main psum pool: sc 2x2 + yT 2 + fill 2x1 = 8 banks ----
            with tc.tile_pool(name="ps2", bufs=1, space="PSUM") as ps2:
                progress = {"v": -1}
                strip_done = set()

                def v_gen():
                    for tt in range(NT):
                        ps = ps2.tile([128, 512], F32, tag="fill", bufs=2, name="vps")
                        for cc in range(NCC):
                            nc.tensor.matmul(
                                ps[:],
                                lhsT=xT[cc][:, tt * 128:(tt + 1) * 128],
                                rhs=wv[:, cc, :],
                                start=(cc == 0),
                                stop=(cc == NCC - 1),
                            )
                            yield
                        nc.vector.tensor_add(
                            v_aug[:, tt, :, 0:D],
                            ps[:].rearrange("p (h d) -> p h d", h=HPC),
                            bv[:].rearrange("p (h d) -> p h d", h=HPC),
                        )
                        progress["v"] = tt
                        yield

                def strip_gen(p):
                    for jt in (p, 4 + p):
                        w_jt = wjp.tile(
                            [128, NCC, 128], F32R, tag="wjt", name=f"w{jt}"
                        )
                        nc.sync.dma_start(
                            w_jt[:],
                            w_d.ap()[:, jt * 128:(jt + 1) * 128].rearrange(
                                "(cc p) j -> p cc j", p=128
                            ),
                        )
                        q = qkp.tile([128, T], F32R, tag="qkT", name=f"qkT{jt}")
                        qkT[jt] = q
                        for tck in range(4):
                            ps = ps2.tile(
                                [128, 512], F32, tag="fill", bufs=2, name="sps"
                            )
                            for cc in range(NCC):
                                nc.tensor.matmul(
                                    ps[:],
                                    lhsT=w_jt[:, cc, :],
                                    rhs=xT[cc][:, tck * 512:(tck + 1) * 512],
                                    start=(cc == 0),
                                    stop=(cc == NCC - 1),
                                )
                                yield
                            nc.vector.tensor_scalar_add(
                                q[:, tck * 512:(tck + 1) * 512],
                                ps[:],
                                bqk[:, jt:jt + 1],
                            )
                            yield
                    strip_done.add(p)

                def p4_gen(ci_lo, ci_hi):
                    for ci in range(ci_lo, ci_hi):
                        for nck in range(2):
                            po = ps2.tile(
                                [128, 512], F32, tag="fill", bufs=2, name="po"
                            )
                            for ki, kc in enumerate((3, 0, 1, 2)):
                                nc.tensor.matmul(
                                    po[:],
                                    lhsT=yT_sb[kc][:, ci * 128:(ci + 1) * 128],
                                    rhs=wp_sb[:, kc, nck * 512:(nck + 1) * 512],
                                    start=(ki == 0),
                                    stop=(ki == 3),
                                )
                                yield
                            o = osbp.tile([128, 512], F32, tag="osb", name="osb")
                            nc.vector.tensor_copy(o[:], po[:])
                            nc.sync.dma_start(
                                out_d.ap()[
                                    ci * 128:(ci + 1) * 128,
                                    nck * 512:(nck + 1) * 512,
                                ],
                                o[:],
                            )
                            yield

                fillers = deque([v_gen(), strip_gen(1), strip_gen(2), strip_gen(3)])

                def pump(n):
                    while n > 0 and fillers:
                        try:
                            next(fillers[0])
                            n -= 1
                        except StopIteration:
                            fillers.popleft()

                def drain_fillers():
                    while fillers:
                        try:
                            next(fillers[0])
                        except StopIteration:
                            fillers.popleft()

                def pump_until(cond):
                    while not cond() and fillers:
                        try:
                            next(fillers[0])
                        except StopIteration:
                            fillers.popleft()

                # ---- P2: attention, baseline per-head structure ----
                for h in range(HPC):
                    p, hb = h // 2, h % 2
                    if hb == 0 and p > 0:
                        pump_until(lambda: p in strip_done)
                    jq, jk = p, 4 + p
                    off = 64 * hb
                    kp = kpad[hb]
                    nc.vector.tensor_copy(
                        kp[off:off + 64, :], qkT[jk][off:off + 64, :]
                    )
                    for half in range(2):
                        if h == HPC - 1 and half == 1:
                            # yT_sb[0..3] half 0 all normalized by now
                            fillers.append(p4_gen(0, 8))
                        ilo, ihi = half * HALF, (half + 1) * HALF
                        cjmax = ihi // 128
                        yT_ps = ps2.tile([D + 1, HALF], F32, tag="yT", name="yTps")

                        def emit_scores_exp(cj):
                            i0 = cj * 128
                            s0 = max(i0, ilo)
                            sc = ps2.tile(
                                [128, HALF], F32, tag="sc", bufs=2, name="sc"
                            )
                            s = s0
                            while s < ihi:
                                e = min(s + 512 - s % 512, ihi)
                                nc.tensor.matmul(
                                    sc[:, s - ilo:e - ilo],
                                    lhsT=kp[:, i0:i0 + 128],
                                    rhs=qkT[jq][:, s:e],
                                    start=True,
                                    stop=True,
                                    skip_group_check=True,
                                )
                                s = e
                            ex = expp.tile([128, HALF], BF16, tag="exp", name="ex")
                            nc.scalar.activation(
                                ex[:, :ihi - s0],
                                sc[:, s0 - ilo:ihi - ilo],
                                mybir.ActivationFunctionType.Exp,
                            )
                            if i0 >= ilo:
                                nc.vector.tensor_mul(
                                    ex[:, 0:128], ex[:, 0:128], mask01[:]
                                )
                            return ex

                        def emit_av(cj, ex):
                            i0 = cj * 128
                            s0 = max(i0, ilo)
                            s = s0
                            while s < ihi:
                                e = min(s + 512 - s % 512, ihi)
                                nc.tensor.matmul(
                                    yT_ps[:, s - ilo:e - ilo],
                                    lhsT=v_aug[:, cj, h, :],
                                    rhs=ex[:, s - s0:e - s0],
                                    start=(cj == 0),
                                    stop=(cj == cjmax - 1),
                                    skip_group_check=True,
                                )
                                s = e

                        prev = None
                        for cj in range(cjmax):
                            pump(3)
                            if prev is not None:
                                pump_until(lambda: progress["v"] >= prev[0])
                                emit_av(*prev)
                            prev = (cj, emit_scores_exp(cj))
                        pump_until(lambda: progress["v"] >= prev[0])
                        emit_av(*prev)
                        if h == HPC - 1 and half == 1:
                            drain_fillers()

                        # ---- normalization (gpsimd DMA queue) ----
                        sums_sb = nrm.tile([1, HALF], F32, tag="ssb", name="ssb")
                        nc.vector.tensor_copy(sums_sb[:], yT_ps[D:D + 1, :])
                        s_dram = dramp.tile([1, HALF], F32, tag="sd")
                        nc.gpsimd.dma_start(s_dram[:], sums_sb[:])
                        srb = nrm.tile([128, HALF // 128], F32, tag="srb", name="srb")
                        nc.gpsimd.dma_start(
                            srb[:], s_dram[:].rearrange("o (p f) -> (o p) f", p=128)
                        )
                        rcp = nrm.tile([128, HALF // 128], F32, tag="rcp", name="rcp")
                        nc.vector.reciprocal(rcp[:], srb[:])
                        r_dram = dramp.tile([1, HALF], F32, tag="rd")
                        nc.gpsimd.dma_start(
                            r_dram[:].rearrange("o (p f) -> (o p) f", p=128), rcp[:]
                        )
                        rb_sb = nrm.tile([64, HALF], F32, tag="rb", name="rb")
                        nc.gpsimd.dma_start(
                            rb_sb[:], r_dram[:].to_broadcast((64, HALF))
                        )
                        nc.vector.tensor_mul(
                            yT_sb[p][off:off + 64, ilo:ihi],
                            yT_ps[0:D, :],
                            rb_sb[:],
                        )
                        if h == 0 and half == 0:
                            nc.sync.dma_start(
                                wp_sb[:],
                                wp_d.ap().rearrange("(kc p) n -> p kc n", p=128),
                            )

                # ---- P4 remainder ----
                for g in (p4_gen(8, 16),):
                    for _ in g:
                        pass

    nc.compile()
    _cached_nc = nc
    return nc


def kernel(x, attn_w, attn_b, proj_w, proj_b):
    global LAST_RESULT
    x = np.asarray(x, dtype=np.float32)
    attn_w = np.asarray(attn_w, dtype=np.float32)
    attn_b = np.asarray(attn_b, dtype=np.float32)
    proj_w = np.asarray(proj_w, dtype=np.float32)
    proj_b = np.asarray(proj_b, dtype=np.float32)

    nc = _build()

    mask01 = np.triu(np.ones((128, 128), dtype=np.float32))  # keep j<=i
    in_maps = []
    for core in range(NC_CORES):
        b, hg = core // 2, core % 2
        qs = slice(hg * 512, hg * 512 + 512)
        ks = slice(C + hg * 512, C + hg * 512 + 512)
        vs = slice(2 * C + hg * 512, 2 * C + hg * 512 + 512)
        w_c = np.concatenate(
            [attn_w[:, qs], attn_w[:, ks], attn_w[:, vs]], axis=1
        )
        in_maps.append(
            {
                "xT": np.ascontiguousarray(x[b].T),
                "w": np.ascontiguousarray(w_c),
                "wp": np.ascontiguousarray(
                    proj_w[hg * 512:hg * 512 + 512, :]
                ).astype(ml_dtypes.bfloat16),
                "bqk": np.ascontiguousarray(
                    np.concatenate([attn_b[qs], attn_b[ks]]).reshape(8, 128).T
                ),
                "bv": np.ascontiguousarray(
                    np.broadcast_to(attn_b[vs][None, :], (128, 512))
                ),
                "mask01": mask01,
            }
        )

    res = run_bass_kernel_spmd(
        nc, in_maps, core_ids=list(range(NC_CORES)), trace=TRACE
    )
    LAST_RESULT = res

    out = np.empty((B, T, C), dtype=np.float32)
    for b in range(B):
        out[b] = (
            res.results[2 * b]["partial"]
            + res.results[2 * b + 1]["partial"]
            + proj_b[None, :]
        )
    return out


# revision 15
# speedup vs baseline: 1.2988x; 1.2666x over previous
"""Causal multi-head attention block (B=4, T=2048, C=1024, H=16) on 8 NeuronCores.

Sharding: core c = 2*b + hg handles batch b, head-group hg (8 heads).
Data parallel over B, tensor parallel over heads: qkv weights column-split,
proj weights row-split; each core emits a partial projection output which the
host sums per batch (plus proj bias).

v3 = baseline P2 matmul structure (K=128 zero-padded scores, M=65 ones-column
AV -- LDWEIGHTS fully hidden, 227ns issue pace) + interleaved PE scheduling:
  - pair-0 q/k strips trickle behind the xT input DMAs (PE starts ~5us);
  - the v projection, later q/k strips, and the first half of P4 are PE
    "fillers" pumped into P2's exp-wait gaps (in-order PE queue: fillers are
    emitted BEFORE the dependency-stalled attention MMs of each cj window);
  - normalization bounces ride the idle gpsimd DMA queue; the scale-mul reads
    yT straight from PSUM; yT_sb and wp are bf16 (P4 in bf16).
PSUM: sc [128,1024] x2bufs (4 banks) + yT [65,1024] (2) + fill [128,512]
x2bufs (2) = 8 banks.
"""
import numpy as np
from collections import deque

import ml_dtypes

import concourse.bacc as bacc
import concourse.mybir as mybir
import concourse.tile as tile
from concourse.bass_utils import run_bass_kernel_spmd

B, T, C, H, D = 4, 2048, 1024, 16, 64
NC_CORES = 8
HPC = H // 2          # heads per core = 8
NPAIR = HPC // 2      # head pairs per core = 4
CW = 3 * C // 2       # packed local qkv width = 1536
NT = T // 128         # 16 token tiles
NCC = C // 128        # 8 contraction chunks
HALF = T // 2
F32 = mybir.dt.float32
F32R = mybir.dt.float32r
BF16 = mybir.dt.bfloat16

TRACE = False          # test.py sets True to profile
LAST_RESULT = None     # BassKernelResults of the last run (for test.py)

_cached_nc = None


def _build():
    global _cached_nc
    if _cached_nc is not None:
        return _cached_nc

    nc = bacc.Bacc("TRN2", debug=False)

    xT_d = nc.dram_tensor("xT", [C, T], F32R, kind="ExternalInput")
    w_d = nc.dram_tensor("w", [C, CW], F32R, kind="ExternalInput")
    wp_d = nc.dram_tensor("wp", [C // 2, C], BF16, kind="ExternalInput")
    bqk_d = nc.dram_tensor("bqk", [128, 8], F32, kind="ExternalInput")
    bv_d = nc.dram_tensor("bv", [128, 512], F32, kind="ExternalInput")
    mask01_d = nc.dram_tensor("mask01", [128, 128], F32, kind="ExternalInput")
    out_d = nc.dram_tensor("partial", [T, C], F32, kind="ExternalOutput")

    with tile.TileContext(nc) as tc:
        with (
            tc.tile_pool(name="const", bufs=1) as const,
            tc.tile_pool(name="xp", bufs=1) as xp,
            tc.tile_pool(name="qkp", bufs=5) as qkp,
            tc.tile_pool(name="wvp", bufs=1) as wvp,
            tc.tile_pool(name="wjp", bufs=2) as wjp,
            tc.tile_pool(name="expp", bufs=3) as expp,
            tc.tile_pool(name="nrm", bufs=1) as nrm,
            tc.tile_pool(name="osbp", bufs=2) as osbp,
            tc.tile_pool(name="dramp", bufs=4, space="DRAM") as dramp,
        ):
            # ---- constants ----
            mask01f = const.tile([128, 128], F32)
            nc.sync.dma_start(mask01f[:], mask01_d.ap())
            mask01 = const.tile([128, 128], BF16)
            nc.vector.tensor_copy(mask01[:], mask01f[:])
            bqk = const.tile([128, 8], F32)
            nc.sync.dma_start(bqk[:], bqk_d.ap())
            bv = const.tile([128, 512], F32)
            nc.sync.dma_start(bv[:], bv_d.ap())
            v_aug = const.tile([128, NT, HPC, D + 1], BF16)
            yT_sb = [const.tile([128, T], BF16, name=f"yT{k}") for k in range(NPAIR)]
            wp_sb = const.tile([128, 4, C], BF16)
            kpad = [const.tile([128, T], F32R, name=f"kpad{s}") for s in range(2)]
            nc.vector.memset(kpad[0][:].bitcast(F32), 0.0)
            nc.vector.memset(kpad[1][:].bitcast(F32), 0.0)
            nc.vector.memset(v_aug[:, :, :, D:D + 1], 1.0)

            # qkT strip tiles, allocated lazily (rotation bufs=5)
            qkT = [None] * 8

            # ---- input DMA order: pair-0 weights first, then xT chunks ----
            w_p0 = []
            for jt in (0, 4):
                w_jt = wjp.tile([128, NCC, 128], F32R, tag="wjt", name=f"w{jt}")
                nc.sync.dma_start(
                    w_jt[:],
                    w_d.ap()[:, jt * 128:(jt + 1) * 128].rearrange(
                        "(cc p) j -> p cc j", p=128
                    ),
                )
                w_p0.append(w_jt)
            xT = [xp.tile([128, T], F32R, name=f"xT{i}") for i in range(NCC)]
            for ccu in range(6):
                nc.sync.dma_start(xT[ccu][:], xT_d.ap()[ccu * 128:(ccu + 1) * 128, :])
            wv = wvp.tile([128, NCC, 512], F32R)
            nc.sync.dma_start(
                wv[:], w_d.ap()[:, 1024:1536].rearrange("(cc p) j -> p cc j", p=128)
            )
            for ccu in range(6, NCC):
                nc.sync.dma_start(xT[ccu][:], xT_d.ap()[ccu * 128:(ccu + 1) * 128, :])

            # ---- P1a: pair-0 q/k strips, trickled behind the xT DMAs ----
            with tc.tile_pool(name="ps1", bufs=1, space="PSUM") as ps1:
                ps_strip = [
                    ps1.tile([128, T], F32, tag=f"s{si}", name=f"ps_s{si}")
                    for si in range(2)
                ]
                for wi in range(16):
                    nc.tensor.matmul(
                        ps_strip[0][:, 0:128],
                        lhsT=mask01[:],
                        rhs=mask01[:],
                        start=True,
                        stop=True,
                        skip_group_check=True,
                    )
                for cc in range(NCC):
                    for si in range(2):
                        for tck in range(4):
                            nc.tensor.matmul(
                                ps_strip[si][:, tck * 512:(tck + 1) * 512],
                                lhsT=w_p0[si][:, cc, :],
                                rhs=xT[cc][:, tck * 512:(tck + 1) * 512],
                                start=(cc == 0),
                                stop=(cc == NCC - 1),
                            )
                for si, jt in ((0, 0), (1, 4)):
                    q = qkp.tile([128, T], F32R, tag="qkT", name=f"qkT{jt}")
                    qkT[jt] = q
                    for tck in range(4):
                        nc.scalar.activation(
                            q[:, tck * 512:(tck + 1) * 512],
                            ps_strip[si][:, tck * 512:(tck + 1) * 512],
                            mybir.ActivationFunctionType.Identity,
                            bias=bqk[:, jt:jt + 1],
                        )

            # ---- main psum pool: sc 2x2 + yT 2 + fill 2x1 = 8 banks ----
            with tc.tile_pool(name="ps2", bufs=1, space="PSUM") as ps2:
                progress = {"v": -1}
                strip_done = set()

                def v_gen():
                    for tt in range(NT):
                        ps = ps2.tile([128, 512], F32, tag="fill", bufs=2, name="vps")
                        for cc in range(NCC):
                            nc.tensor.matmul(
                                ps[:],
                                lhsT=xT[cc][:, tt * 128:(tt + 1) * 128],
                                rhs=wv[:, cc, :],
                                start=(cc == 0),
                                stop=(cc == NCC - 1),
                            )
                            yield
                        nc.vector.tensor_add(
                            v_aug[:, tt, :, 0:D],
                            ps[:].rearrange("p (h d) -> p h d", h=HPC),
                            bv[:].rearrange("p (h d) -> p h d", h=HPC),
                        )
                        progress["v"] = tt
                        yield

                def strip_gen(p):
                    for jt in (p, 4 + p):
                        w_jt = wjp.tile(
                            [128, NCC, 128], F32R, tag="wjt", name=f"w{jt}"
                        )
                        nc.sync.dma_start(
                            w_jt[:],
                            w_d.ap()[:, jt * 128:(jt + 1) * 128].rearrange(
                                "(cc p) j -> p cc j", p=128
                            ),
                        )
                        q = qkp.tile([128, T], F32R, tag="qkT", name=f"qkT{jt}")
                        qkT[jt] = q
                        for tck in range(4):
                            ps = ps2.tile(
                                [128, 512], F32, tag="fill", bufs=2, name="sps"
                            )
                            for cc in range(NCC):
                                nc.tensor.matmul(
                                    ps[:],
                                    lhsT=w_jt[:, cc, :],
                                    rhs=xT[cc][:, tck * 512:(tck + 1) * 512],
                                    start=(cc == 0),
                                    stop=(cc == NCC - 1),
                                )
                                yield
                            nc.vector.tensor_scalar_add(
                                q[:, tck * 512:(tck + 1) * 512],
                                ps[:],
                                bqk[:, jt:jt + 1],
                            )
                            yield
                    strip_done.add(p)

                def p4_gen(ci_lo, ci_hi):
                    for ci in range(ci_lo, ci_hi):
                        for nck in range(2):
                            po = ps2.tile(
                                [128, 512], F32, tag="fill", bufs=2, name="po"
                            )
                            for ki, kc in enumerate((3, 0, 1, 2)):
                                nc.tensor.matmul(
                                    po[:],
                                    lhsT=yT_sb[kc][:, ci * 128:(ci + 1) * 128],
                                    rhs=wp_sb[:, kc, nck * 512:(nck + 1) * 512],
                                    start=(ki == 0),
                                    stop=(ki == 3),
                                )
                                yield
                            o = osbp.tile([128, 512], F32, tag="osb", name="osb")
                            nc.vector.tensor_copy(o[:], po[:])
                            eng = nc.sync if (ci + nck) % 2 == 0 else nc.scalar
                            eng.dma_start(
                                out_d.ap()[
                                    ci * 128:(ci + 1) * 128,
                                    nck * 512:(nck + 1) * 512,
                                ],
                                o[:],
                            )
                            yield

                fillers = deque([v_gen(), strip_gen(1), strip_gen(2), strip_gen(3)])

                def pump(n):
                    while n > 0 and fillers:
                        try:
                            next(fillers[0])
                            n -= 1
                        except StopIteration:
                            fillers.popleft()

                def drain_fillers():
                    while fillers:
                        try:
                            next(fillers[0])
                        except StopIteration:
                            fillers.popleft()

                def pump_until(cond):
                    while not cond() and fillers:
                        try:
                            next(fillers[0])
                        except StopIteration:
                            fillers.popleft()

                # ---- P2: attention, baseline per-head structure ----
                for h in range(HPC):
                    p, hb = h // 2, h % 2
                    if hb == 0 and p > 0:
                        pump_until(lambda: p in strip_done)
                    jq, jk = p, 4 + p
                    off = 64 * hb
                    kp = kpad[hb]
                    nc.vector.tensor_copy(
                        kp[off:off + 64, :], qkT[jk][off:off + 64, :]
                    )
                    for half in range(2):
                        if h == HPC - 1 and half == 1:
                            # yT_sb[0..3] half 0 all normalized by now
                            fillers.append(p4_gen(0, 8))
                        ilo, ihi = half * HALF, (half + 1) * HALF
                        cjmax = ihi // 128
                        yT_ps = ps2.tile([D + 1, HALF], F32, tag="yT", name="yTps")

                        def emit_scores_exp(cj):
                            i0 = cj * 128
                            s0 = max(i0, ilo)
                            sc = ps2.tile(
                                [128, HALF], F32, tag="sc", bufs=2, name="sc"
                            )
                            s = s0
                            while s < ihi:
                                e = min(s + 512 - s % 512, ihi)
                                nc.tensor.matmul(
                                    sc[:, s - ilo:e - ilo],
                                    lhsT=kp[:, i0:i0 + 128],
                                    rhs=qkT[jq][:, s:e],
                                    start=True,
                                    stop=True,
                                    skip_group_check=True,
                                )
                                s = e
                            ex = expp.tile([128, HALF], BF16, tag="exp", name="ex")
                            nc.scalar.activation(
                                ex[:, :ihi - s0],
                                sc[:, s0 - ilo:ihi - ilo],
                                mybir.ActivationFunctionType.Exp,
                            )
                            if i0 >= ilo:
                                nc.vector.tensor_mul(
                                    ex[:, 0:128], ex[:, 0:128], mask01[:]
                                )
                            return ex

                        def emit_av(cj, ex):
                            i0 = cj * 128
                            s0 = max(i0, ilo)
                            s = s0
                            while s < ihi:
                                e = min(s + 512 - s % 512, ihi)
                                nc.tensor.matmul(
                                    yT_ps[:, s - ilo:e - ilo],
                                    lhsT=v_aug[:, cj, h, :],
                                    rhs=ex[:, s - s0:e - s0],
                                    start=(cj == 0),
                                    stop=(cj == cjmax - 1),
                                    skip_group_check=True,
                                )
                                s = e

                        prev = None
                        for cj in range(cjmax):
                            pump(2 if h < 4 else 3)
                            if prev is not None:
                                pump_until(lambda: progress["v"] >= prev[0])
                                emit_av(*prev)
                            prev = (cj, emit_scores_exp(cj))
                        pump_until(lambda: progress["v"] >= prev[0])
                        emit_av(*prev)
                        if h == HPC - 1 and half == 1:
                            drain_fillers()

                        # ---- normalization (gpsimd DMA queue) ----
                        sums_sb = nrm.tile([1, HALF], F32, tag="ssb", name="ssb")
                        nc.vector.tensor_copy(sums_sb[:], yT_ps[D:D + 1, :])
                        ynum = nrm.tile([64, HALF], BF16, tag="ynum", name="ynum")
                        nc.vector.tensor_copy(ynum[:], yT_ps[0:D, :])
                        s_dram = dramp.tile([1, HALF], F32, tag="sd")
                        nc.gpsimd.dma_start(s_dram[:], sums_sb[:])
                        srb = nrm.tile([128, HALF // 128], F32, tag="srb", name="srb")
                        nc.gpsimd.dma_start(
                            srb[:], s_dram[:].rearrange("o (p f) -> (o p) f", p=128)
                        )
                        rcp = nrm.tile([128, HALF // 128], F32, tag="rcp", name="rcp")
                        nc.vector.reciprocal(rcp[:], srb[:])
                        r_dram = dramp.tile([1, HALF], F32, tag="rd")
                        nc.gpsimd.dma_start(
                            r_dram[:].rearrange("o (p f) -> (o p) f", p=128), rcp[:]
                        )
                        rb_sb = nrm.tile([64, HALF], F32, tag="rb", name="rb")
                        nc.gpsimd.dma_start(
                            rb_sb[:], r_dram[:].to_broadcast((64, HALF))
                        )
                        nc.vector.tensor_mul(
                            yT_sb[p][off:off + 64, ilo:ihi],
                            ynum[:],
                            rb_sb[:],
                        )
                        if h == 0 and half == 0:
                            nc.sync.dma_start(
                                wp_sb[:],
                                wp_d.ap().rearrange("(kc p) n -> p kc n", p=128),
                            )

                # ---- P4 remainder ----
                for g in (p4_gen(8, 16),):
                    for _ in g:
                        pass

    nc.compile()
    _cached_nc = nc
    return nc


def kernel(x, attn_w, attn_b, proj_w, proj_b):
    global LAST_RESULT
    x = np.asarray(x, dtype=np.float32)
    attn_w = np.asarray(attn_w, dtype=np.float32)
    attn_b = np.asarray(attn_b, dtype=np.float32)
    proj_w = np.asarray(proj_w, dtype=np.float32)
    proj_b = np.asarray(proj_b, dtype=np.float32)

    nc = _build()

    mask01 = np.triu(np.ones((128, 128), dtype=np.float32))  # keep j<=i
    in_maps = []
    for core in range(NC_CORES):
        b, hg = core // 2, core % 2
        qs = slice(hg * 512, hg * 512 + 512)
        ks = slice(C + hg * 512, C + hg * 512 + 512)
        vs = slice(2 * C + hg * 512, 2 * C + hg * 512 + 512)
        w_c = np.concatenate(
            [attn_w[:, qs], attn_w[:, ks], attn_w[:, vs]], axis=1
        )
        in_maps.append(
            {
                "xT": np.ascontiguousarray(x[b].T),
                "w": np.ascontiguousarray(w_c),
                "wp": np.ascontiguousarray(
                    proj_w[hg * 512:hg * 512 + 512, :]
                ).astype(ml_dtypes.bfloat16),
                "bqk": np.ascontiguousarray(
                    np.concatenate([attn_b[qs], attn_b[ks]]).reshape(8, 128).T
                ),
                "bv": np.ascontiguousarray(
                    np.broadcast_to(attn_b[vs][None, :], (128, 512))
                ),
                "mask01": mask01,
            }
        )

    res = run_bass_kernel_spmd(
        nc, in_maps, core_ids=list(range(NC_CORES)), trace=TRACE
    )
    LAST_RESULT = res

    out = np.empty((B, T, C), dtype=np.float32)
    for b in range(B):
        out[b] = (
            res.results[2 * b]["partial"]
            + res.results[2 * b + 1]["partial"]
            + proj_b[None, :]
        )
    return out


# revision 16
# speedup vs baseline: 1.2990x; 1.0001x over previous
"""Causal multi-head attention block (B=4, T=2048, C=1024, H=16) on 8 NeuronCores.

Sharding: core c = 2*b + hg handles batch b, head-group hg (8 heads).
Data parallel over B, tensor parallel over heads: qkv weights column-split,
proj weights row-split; each core emits a partial projection output which the
host sums per batch (plus proj bias).

v3 = baseline P2 matmul structure (K=128 zero-padded scores, M=65 ones-column
AV -- LDWEIGHTS fully hidden, 227ns issue pace) + interleaved PE scheduling:
  - pair-0 q/k strips trickle behind the xT input DMAs (PE starts ~5us);
  - the v projection, later q/k strips, and the first half of P4 are PE
    "fillers" pumped into P2's exp-wait gaps (in-order PE queue: fillers are
    emitted BEFORE the dependency-stalled attention MMs of each cj window);
  - normalization bounces ride the idle gpsimd DMA queue; the scale-mul reads
    yT straight from PSUM; yT_sb and wp are bf16 (P4 in bf16).
PSUM: sc [128,1024] x2bufs (4 banks) + yT [65,1024] (2) + fill [128,512]
x2bufs (2) = 8 banks.
"""
import numpy as np
from collections import deque

import ml_dtypes

import concourse.bacc as bacc
import concourse.mybir as mybir
import concourse.tile as tile
from concourse.bass_utils import run_bass_kernel_spmd

B, T, C, H, D = 4, 2048, 1024, 16, 64
NC_CORES = 8
HPC = H // 2          # heads per core = 8
NPAIR = HPC // 2      # head pairs per core = 4
CW = 3 * C // 2       # packed local qkv width = 1536
NT = T // 128         # 16 token tiles
NCC = C // 128        # 8 contraction chunks
HALF = T // 2
F32 = mybir.dt.float32
F32R = mybir.dt.float32r
BF16 = mybir.dt.bfloat16

TRACE = False          # test.py sets True to profile
LAST_RESULT = None     # BassKernelResults of the last run (for test.py)

_cached_nc = None


def _build():
    global _cached_nc
    if _cached_nc is not None:
        return _cached_nc

    nc = bacc.Bacc("TRN2", debug=False)

    xT_d = nc.dram_tensor("xT", [C, T], F32R, kind="ExternalInput")
    w_d = nc.dram_tensor("w", [C, CW], F32R, kind="ExternalInput")
    wp_d = nc.dram_tensor("wp", [C // 2, C], BF16, kind="ExternalInput")
    bqk_d = nc.dram_tensor("bqk", [128, 8], F32, kind="ExternalInput")
    bv_d = nc.dram_tensor("bv", [128, 512], F32, kind="ExternalInput")
    mask01_d = nc.dram_tensor("mask01", [128, 128], F32, kind="ExternalInput")
    out_d = nc.dram_tensor("partial", [T, C], F32, kind="ExternalOutput")

    with tile.TileContext(nc) as tc:
        with (
            tc.tile_pool(name="const", bufs=1) as const,
            tc.tile_pool(name="xp", bufs=1) as xp,
            tc.tile_pool(name="qkp", bufs=5) as qkp,
            tc.tile_pool(name="wvp", bufs=1) as wvp,
            tc.tile_pool(name="wjp", bufs=2) as wjp,
            tc.tile_pool(name="expp", bufs=3) as expp,
            tc.tile_pool(name="nrm", bufs=1) as nrm,
            tc.tile_pool(name="osbp", bufs=2) as osbp,
            tc.tile_pool(name="dramp", bufs=4, space="DRAM") as dramp,
        ):
            # ---- constants ----
            mask01f = const.tile([128, 128], F32)
            nc.sync.dma_start(mask01f[:], mask01_d.ap())
            mask01 = const.tile([128, 128], BF16)
            nc.vector.tensor_copy(mask01[:], mask01f[:])
            bqk = const.tile([128, 8], F32)
            nc.sync.dma_start(bqk[:], bqk_d.ap())
            bv = const.tile([128, 512], F32)
            nc.sync.dma_start(bv[:], bv_d.ap())
            v_aug = const.tile([128, NT, HPC, D + 1], BF16)
            yT_sb = [const.tile([128, T], BF16, name=f"yT{k}") for k in range(NPAIR)]
            wp_sb = const.tile([128, 4, C], BF16)
            kpad = [const.tile([128, T], F32R, name=f"kpad{s}") for s in range(2)]
            nc.vector.memset(kpad[0][:].bitcast(F32), 0.0)
            nc.vector.memset(kpad[1][:].bitcast(F32), 0.0)
            nc.vector.memset(v_aug[:, :, :, D:D + 1], 1.0)

            # qkT strip tiles, allocated lazily (rotation bufs=5)
            qkT = [None] * 8

            # ---- input DMA order: pair-0 weights first, then xT chunks ----
            w_p0 = []
            for jt in (0, 4):
                w_jt = wjp.tile([128, NCC, 128], F32R, tag="wjt", name=f"w{jt}")
                nc.sync.dma_start(
                    w_jt[:],
                    w_d.ap()[:, jt * 128:(jt + 1) * 128].rearrange(
                        "(cc p) j -> p cc j", p=128
                    ),
                )
                w_p0.append(w_jt)
            xT = [xp.tile([128, T], F32R, name=f"xT{i}") for i in range(NCC)]
            for ccu in range(6):
                nc.sync.dma_start(xT[ccu][:], xT_d.ap()[ccu * 128:(ccu + 1) * 128, :])
            wv = wvp.tile([128, NCC, 512], F32R)
            nc.sync.dma_start(
                wv[:], w_d.ap()[:, 1024:1536].rearrange("(cc p) j -> p cc j", p=128)
            )
            for ccu in range(6, NCC):
                nc.sync.dma_start(xT[ccu][:], xT_d.ap()[ccu * 128:(ccu + 1) * 128, :])

            # ---- P1a: pair-0 q/k strips, trickled behind the xT DMAs ----
            with tc.tile_pool(name="ps1", bufs=1, space="PSUM") as ps1:
                ps_strip = [
                    ps1.tile([128, T], F32, tag=f"s{si}", name=f"ps_s{si}")
                    for si in range(2)
                ]
                for wi in range(16):
                    nc.tensor.matmul(
                        ps_strip[0][:, 0:128],
                        lhsT=mask01[:],
                        rhs=mask01[:],
                        start=True,
                        stop=True,
                        skip_group_check=True,
                    )
                for cc in range(NCC):
                    for si in range(2):
                        for tck in range(4):
                            nc.tensor.matmul(
                                ps_strip[si][:, tck * 512:(tck + 1) * 512],
                                lhsT=w_p0[si][:, cc, :],
                                rhs=xT[cc][:, tck * 512:(tck + 1) * 512],
                                start=(cc == 0),
                                stop=(cc == NCC - 1),
                            )
                for si, jt in ((0, 0), (1, 4)):
                    q = qkp.tile([128, T], F32R, tag="qkT", name=f"qkT{jt}")
                    qkT[jt] = q
                    for tck in range(4):
                        nc.vector.tensor_scalar_add(
                            q[:, tck * 512:(tck + 1) * 512],
                            ps_strip[si][:, tck * 512:(tck + 1) * 512],
                            bqk[:, jt:jt + 1],
                        )

            # ---- main psum pool: sc 2x2 + yT 2 + fill 2x1 = 8 banks ----
            with tc.tile_pool(name="ps2", bufs=1, space="PSUM") as ps2:
                progress = {"v": -1}
                strip_done = set()

                def v_gen():
                    for tt in range(NT):
                        ps = ps2.tile([128, 512], F32, tag="fill", bufs=2, name="vps")
                        for cc in range(NCC):
                            nc.tensor.matmul(
                                ps[:],
                                lhsT=xT[cc][:, tt * 128:(tt + 1) * 128],
                                rhs=wv[:, cc, :],
                                start=(cc == 0),
                                stop=(cc == NCC - 1),
                            )
                            yield
                        nc.vector.tensor_add(
                            v_aug[:, tt, :, 0:D],
                            ps[:].rearrange("p (h d) -> p h d", h=HPC),
                            bv[:].rearrange("p (h d) -> p h d", h=HPC),
                        )
                        progress["v"] = tt
                        yield

                def strip_gen(p):
                    for jt in (p, 4 + p):
                        w_jt = wjp.tile(
                            [128, NCC, 128], F32R, tag="wjt", name=f"w{jt}"
                        )
                        nc.sync.dma_start(
                            w_jt[:],
                            w_d.ap()[:, jt * 128:(jt + 1) * 128].rearrange(
                                "(cc p) j -> p cc j", p=128
                            ),
                        )
                        q = qkp.tile([128, T], F32R, tag="qkT", name=f"qkT{jt}")
                        qkT[jt] = q
                        for tck in range(4):
                            ps = ps2.tile(
                                [128, 512], F32, tag="fill", bufs=2, name="sps"
                            )
                            for cc in range(NCC):
                                nc.tensor.matmul(
                                    ps[:],
                                    lhsT=w_jt[:, cc, :],
                                    rhs=xT[cc][:, tck * 512:(tck + 1) * 512],
                                    start=(cc == 0),
                                    stop=(cc == NCC - 1),
                                )
                                yield
                            nc.vector.tensor_scalar_add(
                                q[:, tck * 512:(tck + 1) * 512],
                                ps[:],
                                bqk[:, jt:jt + 1],
                            )
                            yield
                    strip_done.add(p)

                def p4_gen(ci_lo, ci_hi):
                    for ci in range(ci_lo, ci_hi):
                        for nck in range(2):
                            po = ps2.tile(
                                [128, 512], F32, tag="fill", bufs=2, name="po"
                            )
                            for ki, kc in enumerate((3, 0, 1, 2)):
                                nc.tensor.matmul(
                                    po[:],
                                    lhsT=yT_sb[kc][:, ci * 128:(ci + 1) * 128],
                                    rhs=wp_sb[:, kc, nck * 512:(nck + 1) * 512],
                                    start=(ki == 0),
                                    stop=(ki == 3),
                                )
                                yield
                            o = osbp.tile([128, 512], F32, tag="osb", name="osb")
                            nc.vector.tensor_copy(o[:], po[:])
                            eng = nc.sync if (ci + nck) % 2 == 0 else nc.scalar
                            eng.dma_start(
                                out_d.ap()[
                                    ci * 128:(ci + 1) * 128,
                                    nck * 512:(nck + 1) * 512,
                                ],
                                o[:],
                            )
                            yield

                fillers = deque([v_gen(), strip_gen(1), strip_gen(2), strip_gen(3)])

                def pump(n):
                    while n > 0 and fillers:
                        try:
                            next(fillers[0])
                            n -= 1
                        except StopIteration:
                            fillers.popleft()

                def drain_fillers():
                    while fillers:
                        try:
                            next(fillers[0])
                        except StopIteration:
                            fillers.popleft()

                def pump_until(cond):
                    while not cond() and fillers:
                        try:
                            next(fillers[0])
                        except StopIteration:
                            fillers.popleft()

                # ---- P2: attention, baseline per-head structure ----
                for h in range(HPC):
                    p, hb = h // 2, h % 2
                    if hb == 0 and p > 0:
                        pump_until(lambda: p in strip_done)
                    jq, jk = p, 4 + p
                    off = 64 * hb
                    kp = kpad[hb]
                    nc.vector.tensor_copy(
                        kp[off:off + 64, :], qkT[jk][off:off + 64, :]
                    )
                    for half in range(2):
                        if h == HPC - 1 and half == 1:
                            # yT_sb[0..3] half 0 all normalized by now
                            fillers.append(p4_gen(0, 8))
                        ilo, ihi = half * HALF, (half + 1) * HALF
                        cjmax = ihi // 128
                        yT_ps = ps2.tile([D + 1, HALF], F32, tag="yT", name="yTps")

                        def emit_scores_exp(cj):
                            i0 = cj * 128
                            s0 = max(i0, ilo)
                            sc = ps2.tile(
                                [128, HALF], F32, tag="sc", bufs=2, name="sc"
                            )
                            s = s0
                            while s < ihi:
                                e = min(s + 512 - s % 512, ihi)
                                nc.tensor.matmul(
                                    sc[:, s - ilo:e - ilo],
                                    lhsT=kp[:, i0:i0 + 128],
                                    rhs=qkT[jq][:, s:e],
                                    start=True,
                                    stop=True,
                                    skip_group_check=True,
                                )
                                s = e
                            ex = expp.tile([128, HALF], BF16, tag="exp", name="ex")
                            nc.scalar.activation(
                                ex[:, :ihi - s0],
                                sc[:, s0 - ilo:ihi - ilo],
                                mybir.ActivationFunctionType.Exp,
                            )
                            if i0 >= ilo:
                                nc.vector.tensor_mul(
                                    ex[:, 0:128], ex[:, 0:128], mask01[:]
                                )
                            return ex

                        def emit_av(cj, ex):
                            i0 = cj * 128
                            s0 = max(i0, ilo)
                            s = s0
                            while s < ihi:
                                e = min(s + 512 - s % 512, ihi)
                                nc.tensor.matmul(
                                    yT_ps[:, s - ilo:e - ilo],
                                    lhsT=v_aug[:, cj, h, :],
                                    rhs=ex[:, s - s0:e - s0],
                                    start=(cj == 0),
                                    stop=(cj == cjmax - 1),
                                    skip_group_check=True,
                                )
                                s = e

                        prev = None
                        for cj in range(cjmax):
                            pump((1, 1, 2, 2, 3, 3, 3, 3)[h])
                            if prev is not None:
                                pump_until(lambda: progress["v"] >= prev[0])
                                emit_av(*prev)
                            prev = (cj, emit_scores_exp(cj))
                        pump_until(lambda: progress["v"] >= prev[0])
                        emit_av(*prev)
                        if h == HPC - 1 and half == 1:
                            drain_fillers()

                        # ---- normalization (gpsimd DMA queue) ----
                        sums_sb = nrm.tile([1, HALF], F32, tag="ssb", name="ssb")
                        nc.vector.tensor_copy(sums_sb[:], yT_ps[D:D + 1, :])
                        ynum = nrm.tile([64, HALF], BF16, tag="ynum", name="ynum")
                        nc.vector.tensor_copy(ynum[:], yT_ps[0:D, :])
                        s_dram = dramp.tile([1, HALF], F32, tag="sd")
                        nc.gpsimd.dma_start(s_dram[:], sums_sb[:])
                        srb = nrm.tile([128, HALF // 128], F32, tag="srb", name="srb")
                        nc.gpsimd.dma_start(
                            srb[:], s_dram[:].rearrange("o (p f) -> (o p) f", p=128)
                        )
                        rcp = nrm.tile([128, HALF // 128], F32, tag="rcp", name="rcp")
                        nc.vector.reciprocal(rcp[:], srb[:])
                        r_dram = dramp.tile([1, HALF], F32, tag="rd")
                        nc.gpsimd.dma_start(
                            r_dram[:].rearrange("o (p f) -> (o p) f", p=128), rcp[:]
                        )
                        rb_sb = nrm.tile([64, HALF], F32, tag="rb", name="rb")
                        nc.gpsimd.dma_start(
                            rb_sb[:], r_dram[:].to_broadcast((64, HALF))
                        )
                        nc.vector.tensor_mul(
                            yT_sb[p][off:off + 64, ilo:ihi],
                            ynum[:],
                            rb_sb[:],
                        )
                        if h == 0 and half == 0:
                            nc.sync.dma_start(
                                wp_sb[:],
                                wp_d.ap().rearrange("(kc p) n -> p kc n", p=128),
                            )

                # ---- P4 remainder ----
                for g in (p4_gen(8, 16),):
                    for _ in g:
                        pass

    nc.compile()
    _cached_nc = nc
    return nc


def kernel(x, attn_w, attn_b, proj_w, proj_b):
    global LAST_RESULT
    x = np.asarray(x, dtype=np.float32)
    attn_w = np.asarray(attn_w, dtype=np.float32)
    attn_b = np.asarray(attn_b, dtype=np.float32)
    proj_w = np.asarray(proj_w, dtype=np.float32)
    proj_b = np.asarray(proj_b, dtype=np.float32)

    nc = _build()

    mask01 = np.triu(np.ones((128, 128), dtype=np.float32))  # keep j<=i
    in_maps = []
    for core in range(NC_CORES):
        b, hg = core // 2, core % 2
        qs = slice(hg * 512, hg * 512 + 512)
        ks = slice(C + hg * 512, C + hg * 512 + 512)
        vs = slice(2 * C + hg * 512, 2 * C + hg * 512 + 512)
        w_c = np.concatenate(
            [attn_w[:, qs], attn_w[:, ks], attn_w[:, vs]], axis=1
        )
        in_maps.append(
            {
                "xT": np.ascontiguousarray(x[b].T),
                "w": np.ascontiguousarray(w_c),
                "wp": np.ascontiguousarray(
                    proj_w[hg * 512:hg * 512 + 512, :]
                ).astype(ml_dtypes.bfloat16),
                "bqk": np.ascontiguousarray(
                    np.concatenate([attn_b[qs], attn_b[ks]]).reshape(8, 128).T
                ),
                "bv": np.ascontiguousarray(
                    np.broadcast_to(attn_b[vs][None, :], (128, 512))
                ),
                "mask01": mask01,
            }
        )

    res = run_bass_kernel_spmd(
        nc, in_maps, core_ids=list(range(NC_CORES)), trace=TRACE
    )
    LAST_RESULT = res

    out = np.empty((B, T, C), dtype=np.float32)
    for b in range(B):
        out[b] = (
            res.results[2 * b]["partial"]
            + res.results[2 * b + 1]["partial"]
            + proj_b[None, :]
        )
    return out
